# revision 25
# baseline (speedup 1.0000x reference)
"""Trainium2 Bass kernel for AttentionWithEncoderKV (stage-2 distill attention).

Contract: kernel(**inputs) takes FULL unsharded numpy inputs and returns the
FULL output tuple (out, distill_loss), matching the jax reference:

    qkv = x @ qkv_w.T + qkv_b                       (B, N, 3, H, hd)
    x_out   = softmax(q*sc @ k^T) @ v               (own attention)
    o_star  = softmax(q*sc @ ek^T) @ ev             (encoder attention)
    loss    = mean((x_out - o_star)^2)              (stage 2 only)
    out     = x_out @ proj_w.T + proj_b

Sharding (8 cores): core c -> sample b = c//2, head-group hg = c%2 (8 of 16
heads).  Each core computes its heads' QKV, both attentions, a distill-loss
partial and a partial output projection (sum over its heads).  The host adds
the two per-sample projection partials (+bias) and the loss partials.

Device-side layout choices (all matmul operands at partition base 0):
 - host pre-transposes x, qkv_w, proj_w, enc_k so no on-device transposes.
 - scores are computed transposed (n_k on partitions) so P@V needs no
   transpose; V carries a leading ones-column so the PV matmul also produces
   softmax denominators in row 0; proj_w gets a matching zero row.
 - fp32r matmuls (full PE rate at free-dim >= 256, ~1.6e-4 component error).
 - per-head q^T/k^T slots are filled from the packed QKV output via
   SBUF->SBUF DMA (compute engines cannot shift partition bases; DMA can).
"""

import math

import numpy as np

_B, _N, _D, _H, _HD = 4, 1024, 1152, 16, 72
_NH = _H // 2            # heads per core
_NT = _N // 128          # token tiles
_KD = _D // 128          # contraction d-tiles
_SC = _HD ** -0.5
_NCORES = 8

_CACHE = {}
_DBG = frozenset()  # debug kill-switches: no_attn, no_norm, no_loss, no_projB
_SCOPES = []        # (instruction-id watermark, label) markers for profiling


def _pieces_by_jt():
    """Split the 16 interleaved 72-channel half-slots (q0,k0,q1,k1,...) of the
    packed 1152-channel QKV output into per-128-tile contiguous pieces.

    Returns {jt: [(a, length, slot, o)]}: rows [a, a+length) of packed tile jt
    hold rows [o, o+length) of half-slot `slot`."""
    out = {jt: [] for jt in range(_KD)}
    for s in range(16):
        c0, c1 = 72 * s, 72 * s + 72
        jt0, jt1 = c0 // 128, (c1 - 1) // 128
        for jt in range(jt0, jt1 + 1):
            lo, hi = max(c0, 128 * jt), min(c1, 128 * jt + 128)
            out[jt].append((lo - 128 * jt, hi - lo, s, lo - c0))
    return out


def _build(mode, reps=1):
    """Build + compile the SPMD device program. mode: 'both' | 'own' | 'enc'.
    reps > 1 wraps the body in a hardware loop (timing builds only)."""
    import concourse.mybir as mybir
    import concourse.tile as tile
    from concourse import bacc

    f32 = mybir.dt.float32
    f32r = mybir.dt.float32r
    AF = mybir.ActivationFunctionType
    OP = mybir.AluOpType

    use_own = mode in ("own", "both")
    use_enc = mode in ("enc", "both")
    use_loss = mode == "both"

    nc = bacc.Bacc("TRN2", target_bir_lowering=False, debug=False,
                   num_devices=_NCORES)

    xT = nc.dram_tensor("xT", (_D, _N), f32r, kind="ExternalInput")
    wqk = nc.dram_tensor("wqk", (_D, 1152), f32r, kind="ExternalInput")
    wv = nc.dram_tensor("wv", (_D, 576), f32r, kind="ExternalInput")
    bqk = nc.dram_tensor("bqk", (128, _KD), f32, kind="ExternalInput")
    bvb = nc.dram_tensor("bvb", (128, 576), f32, kind="ExternalInput")
    enckT = nc.dram_tensor("enckT", (_HD, _NH, _N), f32r, kind="ExternalInput")
    encv = nc.dram_tensor("encv", (_NH, 128, _NT, 73), f32r, kind="ExternalInput")
    pw = nc.dram_tensor("pw", (128, 5, _D), f32r, kind="ExternalInput")
    outT = nc.dram_tensor("outT", (_D, _N), f32, kind="ExternalOutput")
    lossv = nc.dram_tensor("lossv", (73, _NH), f32, kind="ExternalOutput")

    pieces = _pieces_by_jt()
    # head i's slots complete at jt = i+1; emit its attention two j-tiles
    # later so the evac->repack chain stays off the critical path
    heads_done_at = {jt: [] for jt in range(_KD)}
    for i in range(_NH):
        heads_done_at[min((144 * i + 143) // 128 + 2, _KD - 1)].append(i)

    from contextlib import ExitStack

    with tile.TileContext(nc) as tc:
        rep_ctx = ExitStack()
        if reps > 1:
            rep_ctx.enter_context(tc.For_i(0, reps, 1))
        with rep_ctx, ExitStack() as ctx:
            p_dram = ctx.enter_context(
                tc.tile_pool(name="p_dram", bufs=1, space="DRAM"))
            xod = p_dram.tile([128, 5, _N], f32r, name="xod")
            with ExitStack() as actx:
                def pool(name, bufs, space="SBUF"):
                    return actx.enter_context(
                        tc.tile_pool(name=name, bufs=bufs, space=space))

                p_xt = pool("p_xt", 1)
                p_wv = pool("p_wv", 1)
                p_bias = pool("p_bias", 1)
                p_vaug = pool("p_vaug", 1)
                p_wqk = pool("p_wqk", 3)
                p_qkpk = pool("p_qkpk", 3)
                p_slot = pool("p_slot", 7)
                p_exp = pool("p_exp", 3)
                p_enck = pool("p_enck", 2)
                p_encv = pool("p_encv", 2)
                p_xo = pool("p_xo", 4)
                p_ostar = pool("p_ostar", 2)
                p_oev = pool("p_oev", 3)
                p_rr = pool("p_rr", 1)
                p_rb = pool("p_rb", 1)
                p_diff = pool("p_diff", 1)
                p_sq = pool("p_sq", 1)
                p_loss = pool("p_loss", 1)
                p_mm = pool("p_mm", 3, "PSUM")
                p_ops = pool("p_ops", 1, "PSUM")

                # ---- resident loads (split DMAs so compute can start early)
                bqk_sb = p_bias.tile([128, _KD], f32, name="bqk_sb", tag="bqk")
                nc.sync.dma_start(bqk_sb[:], bqk.ap())
                bvb_sb = p_bias.tile([128, 576], f32, name="bvb_sb", tag="bvb")
                nc.sync.dma_start(bvb_sb[:], bvb.ap())
                xt = p_xt.tile([128, _KD, _N], f32r, name="xt")
                xTr = xT.ap().rearrange("(kt p) n -> p kt n", p=128)
                wv_sb = p_wv.tile([128, _KD, 576], f32r, name="wv_sb")
                wvr = wv.ap().rearrange("(kt p) j -> p kt j", p=128)
                for kt in range(_KD):
                    nc.sync.dma_start(xt[:, kt, :], xTr[:, kt, :])
                    nc.sync.dma_start(wv_sb[:, kt, :], wvr[:, kt, :])

                loss_sb = p_loss.tile([73, _NH], f32, name="loss_sb")
                nc.gpsimd.memset(loss_sb[:], 0.0)

                # ---- v projection into aug layout [p][kb][h][1+hd], col 0 = 1
                _SCOPES.append((nc.next_id(), "v_phase"))
                vaug = p_vaug.tile([128, _NT, _NH, 73], f32r, name="vaug")
                # memset cannot produce f32r; write the ones column (col 0 of
                # every (kb, h) slab) as 0*x + 1 via tensor_scalar instead
                zs = p_bias.tile([128, 64], f32, name="zs", tag="zs")
                nc.vector.memset(zs[:], 0.0)
                nc.vector.tensor_scalar(
                    out=vaug[:, :, :, 0:1],
                    in0=zs[:].rearrange("p (a b c) -> p a b c", b=_NH, c=1),
                    scalar1=0.0, scalar2=1.0,
                    op0=OP.mult, op1=OP.add)
                # ---- per-head slot tiles (created lazily in piece order)
                slots = {}

                def slot_tile(s):
                    if s not in slots:
                        kind = "q" if s % 2 == 0 else "k"
                        slots[s] = p_slot.tile(
                            [72, _N], f32r, name=f"slot_{kind}{s // 2}",
                            tag="slot")
                    return slots[s]

                wslabs = {}

                def load_wslab(jt):
                    w = p_wqk.tile([128, _KD, 128], f32r,
                                   name=f"wslab{jt}", tag="wqk")
                    nc.sync.dma_start(
                        w[:],
                        wqk.ap().rearrange("(kt p) j -> p kt j", p=128)
                        [:, :, jt * 128:(jt + 1) * 128])
                    wslabs[jt] = w

                def emit_qk_jtile(jt):
                    wslab = wslabs.pop(jt)
                    qp = p_mm.tile([128, 1024], f32, name=f"qp{jt}", tag="mm")
                    for kt in range(_KD):
                        for nch in range(2):
                            nc.tensor.matmul(
                                qp[:, nch * 512:(nch + 1) * 512],
                                wslab[:, kt, :],
                                xt[:, kt, nch * 512:(nch + 1) * 512],
                                start=(kt == 0), stop=(kt == _KD - 1))
                    pk = p_qkpk.tile([128, 1024], f32r, name=f"pk{jt}",
                                     tag="qkpk")
                    nc.vector.tensor_scalar_add(pk[:], qp[:],
                                                bqk_sb[:, jt:jt + 1])
                    for (a, ln, s, o) in pieces[jt]:
                        nc.sync.dma_start(slot_tile(s)[o:o + ln, :],
                                          pk[a:a + ln, :])

                load_wslab(0)
                load_wslab(1)
                for nt in range(_NT):
                    if nt == 4:
                        _SCOPES.append((nc.next_id(), "qk_jt0"))
                        emit_qk_jtile(0)
                    vp = p_mm.tile([128, 1024], f32, name=f"vp{nt}", tag="mm")
                    for kt in range(_KD):
                        for half in range(2):
                            nc.tensor.matmul(
                                vp[:, half * 512: half * 512 + 288],
                                xt[:, kt, nt * 128:(nt + 1) * 128],
                                wv_sb[:, kt, half * 288:(half + 1) * 288],
                                start=(kt == 0), stop=(kt == _KD - 1))
                    for half in range(2):
                        dst = vp[:, half * 512: half * 512 + 288]
                        nc.vector.tensor_tensor(
                            out=vaug[:, nt, half * 4: half * 4 + 4, 1:73],
                            in0=dst.rearrange("p (h d) -> p h d", d=72),
                            in1=bvb_sb[:, half * 288:(half + 1) * 288]
                                .rearrange("p (h d) -> p h d", d=72),
                            op=OP.add)

                def attention(i, kT_ap, v_of_kb, out_tile, out_name_hint):
                    """Emit one attention (head i): scores^T, exp, PV(+denom),
                    normalize into out_tile (73, N)."""
                    _SCOPES.append((nc.next_id(), f"attn_{out_name_hint}"))
                    q = slots[2 * i]
                    ops_t = p_ops.tile([73, 1024], f32,
                                       name=f"o_{out_name_hint}", tag="o")
                    for kb in range(_NT):
                        sp = p_mm.tile([128, 1024], f32,
                                       name=f"s_{out_name_hint}_{kb}", tag="mm")
                        for nch in range(2):
                            nc.tensor.matmul(
                                sp[:, nch * 512:(nch + 1) * 512],
                                kT_ap[:, kb * 128:(kb + 1) * 128],
                                q[:, nch * 512:(nch + 1) * 512],
                                start=True, stop=True)
                        ex = p_exp.tile([128, 1024], f32r,
                                        name=f"e_{out_name_hint}_{kb}",
                                        tag="exp")
                        nc.scalar.activation(ex[:], sp[:], AF.Exp, scale=_SC)
                        for nch in range(2):
                            nc.tensor.matmul(
                                ops_t[:, nch * 512:(nch + 1) * 512],
                                v_of_kb(kb),
                                ex[:, nch * 512:(nch + 1) * 512],
                                start=(kb == 0), stop=(kb == _NT - 1))
                    if "no_norm" in _DBG:
                        nc.vector.tensor_copy(out_tile[:], ops_t[:])
                        return
                    oev = p_oev.tile([73, 1024], f32,
                                     name=f"oev_{out_name_hint}", tag="oev")
                    nc.vector.tensor_copy(oev[:], ops_t[:])
                    rr = p_rr.tile([1, 1024], f32,
                                   name=f"rr_{out_name_hint}", tag="rr")
                    nc.vector.reciprocal(rr[:], oev[0:1, :])
                    rb = p_rb.tile([73, 1024], f32,
                                   name=f"rb_{out_name_hint}", tag="rb")
                    nc.gpsimd.partition_broadcast(rb[:], rr[:])
                    nc.vector.tensor_tensor(
                        out=out_tile[:], in0=oev[:], in1=rb[:], op=OP.mult)

                # ---- packed qk projection tiles, interleaved with attention
                for jt in range(1, _KD):
                    _SCOPES.append((nc.next_id(), f"qk_jt{jt}"))
                    if jt + 1 < _KD:
                        load_wslab(jt + 1)
                    emit_qk_jtile(jt)

                    for i in heads_done_at[jt]:
                        if "no_attn" in _DBG:
                            continue
                        ek = ev = None
                        if use_enc:
                            # prefetch encoder K/V before the own attention
                            ek = p_enck.tile([72, _N], f32r, name=f"ek{i}",
                                             tag="enck")
                            nc.sync.dma_start(ek[:], enckT.ap()[:, i, :])
                            ev = p_encv.tile([128, _NT, 73], f32r,
                                             name=f"ev{i}", tag="encv")
                            nc.sync.dma_start(ev[:], encv.ap()[i])
                        xo = None
                        if use_own:
                            xo = p_xo.tile([73, _N], f32r, name=f"xo{i}",
                                           tag="xo")
                            attention(i, slots[2 * i + 1],
                                      lambda kb, i=i: vaug[:, kb, i, :],
                                      xo, f"own{i}")
                        ostar = None
                        if use_enc:
                            dt_enc = f32r if mode == "enc" else f32
                            ostar = p_ostar.tile([73, _N], dt_enc,
                                                 name=f"os{i}", tag="ostar")
                            attention(i, ek[:],
                                      lambda kb, ev=ev: ev[:, kb, :],
                                      ostar, f"enc{i}")
                        if use_loss and "no_loss" not in _DBG:
                            df = p_diff.tile([73, _N], f32, name=f"df{i}",
                                             tag="diff")
                            nc.vector.tensor_tensor(
                                out=df[:], in0=xo[:].bitcast(f32),
                                in1=ostar[:], op=OP.subtract)
                            sq = p_sq.tile([73, _N], f32, name=f"sq{i}",
                                           tag="sq")
                            nc.vector.tensor_tensor(
                                out=sq[:], in0=df[:], in1=df[:], op=OP.mult)
                            nc.vector.reduce_sum(
                                loss_sb[:, i:i + 1], sq[:],
                                axis=mybir.AxisListType.X)
                        # spill rows 1:73 packed at rows [72i, 72i+72)
                        spill = xo if use_own else ostar
                        c0 = 72 * i
                        for kt in range(c0 // 128, (c0 + 71) // 128 + 1):
                            lo = max(c0, 128 * kt)
                            hi = min(c0 + 72, 128 * kt + 128)
                            nc.sync.dma_start(
                                xod[lo - 128 * kt: hi - 128 * kt, kt, :],
                                spill[1 + lo - c0: 1 + hi - c0, :])

                nc.sync.dma_start(lossv.ap(), loss_sb[:])

            # ---- phase B: single-pass projection over the packed spill
            with ExitStack() as bctx:
                p_pw = bctx.enter_context(tc.tile_pool(name="p_pw", bufs=1))
                p_xor = bctx.enter_context(tc.tile_pool(name="p_xor", bufs=1))
                p_pout = bctx.enter_context(tc.tile_pool(name="p_pout", bufs=4))
                p_pps = bctx.enter_context(
                    tc.tile_pool(name="p_pps", bufs=3, space="PSUM"))
                _SCOPES.append((nc.next_id(), "proj"))
                xop = p_xor.tile([128, 5, _N], f32r, name="xop")
                pwp = p_pw.tile([128, 5, _D], f32r, name="pwp")
                for kt in range(5):
                    nc.sync.dma_start(xop[:, kt, :], xod[:, kt, :])
                    nc.sync.dma_start(pwp[:, kt, :], pw.ap()[:, kt, :])
                for ct in range(_KD):
                    pps = p_pps.tile([128, 1024], f32, name=f"pps{ct}",
                                     tag="pps")
                    for kt in range(5):
                        kk = 128 if kt < 4 else 64
                        for nch in range(2):
                            nc.tensor.matmul(
                                pps[:, nch * 512:(nch + 1) * 512],
                                pwp[0:kk, kt, ct * 128:(ct + 1) * 128],
                                xop[0:kk, kt, nch * 512:(nch + 1) * 512],
                                start=(kt == 0), stop=(kt == 4))
                    po = p_pout.tile([128, 1024], f32, name=f"po{ct}",
                                     tag="po")
                    if ct % 2 == 0:
                        nc.vector.tensor_copy(po[:], pps[:])
                    else:
                        nc.scalar.copy(po[:], pps[:])
                    nc.sync.dma_start(
                        outT.ap()[ct * 128:(ct + 1) * 128, :], po[:])

    nc.compile()
    return nc


def _prep_core(x, enc_k, enc_v, wT, qkv_b, pwT_aug, b, hg):
    """Host-side per-core input dict. wT = qkv_w.T (D, 3D); pwT_aug
    (H, 73, D) with zero row 0."""
    heads = range(hg * _NH, hg * _NH + _NH)
    qcols = np.concatenate(
        [np.arange(h * _HD, (h + 1) * _HD) for h in heads])
    # interleaved [q_h, k_h] channel order
    cols = np.empty(2 * len(qcols), np.int64)
    for idx, h in enumerate(heads):
        base = 144 * idx
        cols[base:base + 72] = np.arange(h * _HD, (h + 1) * _HD)
        cols[base + 72:base + 144] = _D + np.arange(h * _HD, (h + 1) * _HD)
    vcols = 2 * _D + qcols

    xTc = np.ascontiguousarray(x[b].T)
    wqk_c = np.ascontiguousarray(wT[:, cols])
    wv_c = np.ascontiguousarray(wT[:, vcols])
    bqk_c = np.ascontiguousarray(qkv_b[cols].reshape(_KD, 128).T)
    bvb_c = np.ascontiguousarray(
        np.broadcast_to(qkv_b[vcols], (128, 576)))
    ek = enc_k[b, list(heads)]                      # (NH, N, hd)
    enckT_c = np.ascontiguousarray(ek.transpose(2, 0, 1))   # (hd, NH, N)
    ev = enc_v[b, list(heads)].reshape(_NH, _NT, 128, _HD)
    encv_c = np.zeros((_NH, 128, _NT, 73), np.float32)
    encv_c[:, :, :, 0] = 1.0
    encv_c[:, :, :, 1:] = ev.transpose(0, 2, 1, 3)
    pw_c = pwT_aug[hg]
    return {
        "xT": xTc, "wqk": wqk_c, "wv": wv_c, "bqk": bqk_c, "bvb": bvb_c,
        "enckT": enckT_c, "encv": encv_c, "pw": pw_c,
    }



def build_in_maps(x, enc_k, enc_v, qkv_w, qkv_b, proj_w):
    """Host-side prep: per-core input dicts for all 8 cores."""
    x = np.asarray(x, np.float32)
    enc_k = np.asarray(enc_k, np.float32)
    enc_v = np.asarray(enc_v, np.float32)
    qkv_w = np.asarray(qkv_w, np.float32)
    qkv_b = np.asarray(qkv_b, np.float32)
    proj_w = np.asarray(proj_w, np.float32)
    wT = np.ascontiguousarray(qkv_w.T)
    pwT = np.asarray(proj_w.T)                       # (j, c)
    # packed per-core proj weights: (128, 5, D), rows = 8 heads x 72, zero pad
    pwT_aug = np.zeros((2, 640, _D), np.float32)
    for hg in range(2):
        pwT_aug[hg, :576] = pwT.reshape(_H, _HD, _D)[
            hg * _NH: hg * _NH + _NH].reshape(576, _D)
    pwT_aug = np.ascontiguousarray(
        pwT_aug.reshape(2, 5, 128, _D).transpose(0, 2, 1, 3))  # (2,128,5,D)
    return [
        _prep_core(x, enc_k, enc_v, wT, qkv_b, pwT_aug, c // 2, c % 2)
        for c in range(_NCORES)
    ]


LAST_EXEC_NS = None


def kernel(x, enc_k, enc_v, qkv_w, qkv_b, proj_w, proj_b, stage):
    global LAST_EXEC_NS
    from concourse.bass_utils import run_bass_kernel_spmd

    x = np.asarray(x, np.float32)
    enc_k = np.asarray(enc_k, np.float32)
    enc_v = np.asarray(enc_v, np.float32)
    qkv_w = np.asarray(qkv_w, np.float32)
    qkv_b = np.asarray(qkv_b, np.float32)
    proj_w = np.asarray(proj_w, np.float32)
    proj_b = np.asarray(proj_b, np.float32)
    stage = int(np.asarray(stage))
    mode = {1: "enc", 2: "both"}.get(stage, "own")

    if mode not in _CACHE:
        _CACHE[mode] = _build(mode)
    nc = _CACHE[mode]

    in_maps = build_in_maps(x, enc_k, enc_v, qkv_w, qkv_b, proj_w)
    res = run_bass_kernel_spmd(nc, in_maps, core_ids=list(range(_NCORES)))
    LAST_EXEC_NS = res.exec_time_ns

    out = np.empty((_B, _N, _D), np.float32)
    for b in range(_B):
        acc = res.results[2 * b]["outT"] + res.results[2 * b + 1]["outT"]
        out[b] = acc.T + proj_b

    if mode == "both":
        tot = sum(float(r["lossv"].sum()) for r in res.results)
        loss = np.float32(tot / (_B * _H * _N * _HD))
    else:
        loss = np.float32(0.0)
    return out, loss


# revision 28
# speedup vs baseline: 1.0367x; 1.0367x over previous
"""Trainium2 Bass kernel for AttentionWithEncoderKV (stage-2 distill attention).

Contract: kernel(**inputs) takes FULL unsharded numpy inputs and returns the
FULL output tuple (out, distill_loss), matching the jax reference:

    qkv = x @ qkv_w.T + qkv_b                       (B, N, 3, H, hd)
    x_out   = softmax(q*sc @ k^T) @ v               (own attention)
    o_star  = softmax(q*sc @ ek^T) @ ev             (encoder attention)
    loss    = mean((x_out - o_star)^2)              (stage 2 only)
    out     = x_out @ proj_w.T + proj_b

Sharding (8 cores): core c -> sample b = c//2, head-group hg = c%2 (8 of 16
heads).  Each core computes its heads' QKV, both attentions, a distill-loss
partial and a partial output projection (sum over its heads).  The host adds
the two per-sample projection partials (+bias) and the loss partials.

Device-side layout choices (all matmul operands at partition base 0):
 - host pre-transposes x, qkv_w, proj_w, enc_k so no on-device transposes.
 - scores are computed transposed (n_k on partitions) so P@V needs no
   transpose; V carries a leading ones-column so the PV matmul also produces
   softmax denominators in row 0; proj_w gets a matching zero row.
 - fp32r matmuls (full PE rate at free-dim >= 256, ~1.6e-4 component error).
 - per-head q^T/k^T slots are filled from the packed QKV output via
   SBUF->SBUF DMA (compute engines cannot shift partition bases; DMA can).
 - softmax denominators: DVE reciprocal of row 0, GPSIMD partition-broadcast,
   one DVE multiply; exp runs on ScalarE with the hd^-0.5 scale fused.
 - normalized per-head outputs spill to DRAM packed (128, 5, N) so the output
   projection runs one K=128-packed pass with 2 psum banks.
 - consecutive matmuls share their stationary operand (kt-outer/nch-inner
   loops) to halve the per-matmul self-weight-load overhead of fp32r.

Measured (8 trn2 cores, this container): relative error 4.1e-4 vs the fp32
jax reference (loss 2.4e-6); ~320-330 us per execution (hardware-loop slope
method); cost-model timeline 232 us with PE busy 183 us (79% occupancy).
"""

import math

import numpy as np

_B, _N, _D, _H, _HD = 4, 1024, 1152, 16, 72
_NH = _H // 2            # heads per core
_NT = _N // 128          # token tiles
_KD = _D // 128          # contraction d-tiles
_SC = _HD ** -0.5
_NCORES = 8

_CACHE = {}
_DBG = frozenset()  # debug kill-switches: no_attn, no_norm, no_loss, no_projB
_SCOPES = []        # (instruction-id watermark, label) markers for profiling


def _pieces_by_jt():
    """Split the 16 interleaved 72-channel half-slots (q0,k0,q1,k1,...) of the
    packed 1152-channel QKV output into per-128-tile contiguous pieces.

    Returns {jt: [(a, length, slot, o)]}: rows [a, a+length) of packed tile jt
    hold rows [o, o+length) of half-slot `slot`."""
    out = {jt: [] for jt in range(_KD)}
    for s in range(16):
        c0, c1 = 72 * s, 72 * s + 72
        jt0, jt1 = c0 // 128, (c1 - 1) // 128
        for jt in range(jt0, jt1 + 1):
            lo, hi = max(c0, 128 * jt), min(c1, 128 * jt + 128)
            out[jt].append((lo - 128 * jt, hi - lo, s, lo - c0))
    return out


def _build(mode, reps=1):
    """Build + compile the SPMD device program. mode: 'both' | 'own' | 'enc'.
    reps > 1 wraps the body in a hardware loop (timing builds only)."""
    import concourse.mybir as mybir
    import concourse.tile as tile
    from concourse import bacc

    f32 = mybir.dt.float32
    f32r = mybir.dt.float32r
    AF = mybir.ActivationFunctionType
    OP = mybir.AluOpType

    use_own = mode in ("own", "both")
    use_enc = mode in ("enc", "both")
    use_loss = mode == "both"

    nc = bacc.Bacc("TRN2", target_bir_lowering=False, debug=False,
                   num_devices=_NCORES)

    xT = nc.dram_tensor("xT", (_D, _N), f32r, kind="ExternalInput")
    wqk = nc.dram_tensor("wqk", (_D, 1152), f32r, kind="ExternalInput")
    wv = nc.dram_tensor("wv", (_D, 576), f32r, kind="ExternalInput")
    bqk = nc.dram_tensor("bqk", (128, _KD), f32, kind="ExternalInput")
    bvb = nc.dram_tensor("bvb", (128, 576), f32, kind="ExternalInput")
    enckT = nc.dram_tensor("enckT", (_HD, _NH, _N), f32r, kind="ExternalInput")
    encv = nc.dram_tensor("encv", (_NH, 128, _NT, 73), f32r, kind="ExternalInput")
    pw = nc.dram_tensor("pw", (128, 5, _D), f32r, kind="ExternalInput")
    outT = nc.dram_tensor("outT", (_D, _N), f32, kind="ExternalOutput")
    lossv = nc.dram_tensor("lossv", (73, _NH), f32, kind="ExternalOutput")

    pieces = _pieces_by_jt()
    # head i's slots complete at jt = i+1; emit its attention two j-tiles
    # later so the evac->repack chain stays off the critical path
    heads_done_at = {jt: [] for jt in range(_KD)}
    for i in range(_NH):
        heads_done_at[min((144 * i + 143) // 128 + 2, _KD - 1)].append(i)

    from contextlib import ExitStack

    with tile.TileContext(nc) as tc:
        rep_ctx = ExitStack()
        if reps > 1:
            rep_ctx.enter_context(tc.For_i(0, reps, 1))
        with rep_ctx, ExitStack() as ctx:
            p_dram = ctx.enter_context(
                tc.tile_pool(name="p_dram", bufs=1, space="DRAM"))
            xod = p_dram.tile([128, 5, _N], f32r, name="xod")
            with ExitStack() as actx:
                def pool(name, bufs, space="SBUF"):
                    return actx.enter_context(
                        tc.tile_pool(name=name, bufs=bufs, space=space))

                p_xt = pool("p_xt", 1)
                p_wv = pool("p_wv", 1)
                p_bias = pool("p_bias", 1)
                p_vaug = pool("p_vaug", 1)
                p_wqk = pool("p_wqk", 3)
                p_qkpk = pool("p_qkpk", 3)
                p_slot = pool("p_slot", 7)
                p_exp = pool("p_exp", 3)
                p_enck = pool("p_enck", 2)
                p_encv = pool("p_encv", 2)
                p_xo = pool("p_xo", 4)
                p_ostar = pool("p_ostar", 2)
                p_oev = pool("p_oev", 3)
                p_rr = pool("p_rr", 1)
                p_rb = pool("p_rb", 1)
                p_diff = pool("p_diff", 1)
                p_sq = pool("p_sq", 1)
                p_loss = pool("p_loss", 1)
                p_mm = pool("p_mm", 3, "PSUM")
                p_ops = pool("p_ops", 1, "PSUM")

                # ---- resident loads (split DMAs so compute can start early)
                bqk_sb = p_bias.tile([128, _KD], f32, name="bqk_sb", tag="bqk")
                nc.sync.dma_start(bqk_sb[:], bqk.ap())
                bvb_sb = p_bias.tile([128, 576], f32, name="bvb_sb", tag="bvb")
                nc.sync.dma_start(bvb_sb[:], bvb.ap())
                xt = p_xt.tile([128, _KD, _N], f32r, name="xt")
                xTr = xT.ap().rearrange("(kt p) n -> p kt n", p=128)
                wv_sb = p_wv.tile([128, _KD, 576], f32r, name="wv_sb")
                wvr = wv.ap().rearrange("(kt p) j -> p kt j", p=128)
                for kt in range(_KD):
                    nc.sync.dma_start(xt[:, kt, :], xTr[:, kt, :])
                    nc.sync.dma_start(wv_sb[:, kt, :], wvr[:, kt, :])

                loss_sb = p_loss.tile([73, _NH], f32, name="loss_sb")
                nc.gpsimd.memset(loss_sb[:], 0.0)

                # ---- v projection into aug layout [p][kb][h][1+hd], col 0 = 1
                _SCOPES.append((nc.next_id(), "v_phase"))
                vaug = p_vaug.tile([128, _NT, _NH, 73], f32r, name="vaug")
                # memset cannot produce f32r; write the ones column (col 0 of
                # every (kb, h) slab) as 0*x + 1 via tensor_scalar instead
                zs = p_bias.tile([128, 64], f32, name="zs", tag="zs")
                nc.vector.memset(zs[:], 0.0)
                nc.vector.tensor_scalar(
                    out=vaug[:, :, :, 0:1],
                    in0=zs[:].rearrange("p (a b c) -> p a b c", b=_NH, c=1),
                    scalar1=0.0, scalar2=1.0,
                    op0=OP.mult, op1=OP.add)
                # ---- per-head slot tiles (created lazily in piece order)
                slots = {}

                def slot_tile(s):
                    if s not in slots:
                        kind = "q" if s % 2 == 0 else "k"
                        slots[s] = p_slot.tile(
                            [72, _N], f32r, name=f"slot_{kind}{s // 2}",
                            tag="slot")
                    return slots[s]

                wslabs = {}

                def load_wslab(jt):
                    w = p_wqk.tile([128, _KD, 128], f32r,
                                   name=f"wslab{jt}", tag="wqk")
                    nc.sync.dma_start(
                        w[:],
                        wqk.ap().rearrange("(kt p) j -> p kt j", p=128)
                        [:, :, jt * 128:(jt + 1) * 128])
                    wslabs[jt] = w

                def emit_qk_jtile(jt):
                    wslab = wslabs.pop(jt)
                    qp = p_mm.tile([128, 1024], f32, name=f"qp{jt}", tag="mm")
                    for kt in range(_KD):
                        for nch in range(2):
                            nc.tensor.matmul(
                                qp[:, nch * 512:(nch + 1) * 512],
                                wslab[:, kt, :],
                                xt[:, kt, nch * 512:(nch + 1) * 512],
                                start=(kt == 0), stop=(kt == _KD - 1))
                    pk = p_qkpk.tile([128, 1024], f32r, name=f"pk{jt}",
                                     tag="qkpk")
                    nc.vector.tensor_scalar_add(pk[:], qp[:],
                                                bqk_sb[:, jt:jt + 1])
                    for (a, ln, s, o) in pieces[jt]:
                        nc.sync.dma_start(slot_tile(s)[o:o + ln, :],
                                          pk[a:a + ln, :])

                load_wslab(0)
                load_wslab(1)
                for nt in range(_NT):
                    if nt == 4:
                        _SCOPES.append((nc.next_id(), "qk_jt0"))
                        emit_qk_jtile(0)
                    vp = p_mm.tile([128, 1024], f32, name=f"vp{nt}", tag="mm")
                    for kt in range(_KD):
                        for half in range(2):
                            nc.tensor.matmul(
                                vp[:, half * 512: half * 512 + 288],
                                xt[:, kt, nt * 128:(nt + 1) * 128],
                                wv_sb[:, kt, half * 288:(half + 1) * 288],
                                start=(kt == 0), stop=(kt == _KD - 1))
                    for half in range(2):
                        dst = vp[:, half * 512: half * 512 + 288]
                        nc.vector.tensor_tensor(
                            out=vaug[:, nt, half * 4: half * 4 + 4, 1:73],
                            in0=dst.rearrange("p (h d) -> p h d", d=72),
                            in1=bvb_sb[:, half * 288:(half + 1) * 288]
                                .rearrange("p (h d) -> p h d", d=72),
                            op=OP.add)

                def attention(i, kT_ap, v_of_kb, out_tile, out_name_hint):
                    """Emit one attention (head i): scores^T, exp, PV(+denom),
                    normalize into out_tile (73, N)."""
                    _SCOPES.append((nc.next_id(), f"attn_{out_name_hint}"))
                    q = slots[2 * i]
                    ops_t = p_ops.tile([73, 1024], f32,
                                       name=f"o_{out_name_hint}", tag="o")
                    for kb in range(_NT):
                        sp = p_mm.tile([128, 1024], f32,
                                       name=f"s_{out_name_hint}_{kb}", tag="mm")
                        for nch in range(2):
                            nc.tensor.matmul(
                                sp[:, nch * 512:(nch + 1) * 512],
                                kT_ap[:, kb * 128:(kb + 1) * 128],
                                q[:, nch * 512:(nch + 1) * 512],
                                start=True, stop=True)
                        ex = p_exp.tile([128, 1024], f32r,
                                        name=f"e_{out_name_hint}_{kb}",
                                        tag="exp")
                        nc.scalar.activation(ex[:], sp[:], AF.Exp, scale=_SC)
                        for nch in range(2):
                            nc.tensor.matmul(
                                ops_t[:, nch * 512:(nch + 1) * 512],
                                v_of_kb(kb),
                                ex[:, nch * 512:(nch + 1) * 512],
                                start=(kb == 0), stop=(kb == _NT - 1))
                    if "no_norm" in _DBG:
                        nc.vector.tensor_copy(out_tile[:], ops_t[:])
                        return
                    oev = p_oev.tile([73, 1024], f32,
                                     name=f"oev_{out_name_hint}", tag="oev")
                    nc.vector.tensor_copy(oev[:], ops_t[:])
                    rr = p_rr.tile([1, 1024], f32,
                                   name=f"rr_{out_name_hint}", tag="rr")
                    nc.vector.reciprocal(rr[:], oev[0:1, :])
                    rb = p_rb.tile([73, 1024], f32,
                                   name=f"rb_{out_name_hint}", tag="rb")
                    nc.gpsimd.partition_broadcast(rb[:], rr[:])
                    nc.vector.tensor_tensor(
                        out=out_tile[:], in0=oev[:], in1=rb[:], op=OP.mult)

                # ---- packed qk projection tiles, interleaved with attention
                for jt in range(1, _KD):
                    _SCOPES.append((nc.next_id(), f"qk_jt{jt}"))
                    if jt + 1 < _KD:
                        load_wslab(jt + 1)
                    emit_qk_jtile(jt)

                    for i in heads_done_at[jt]:
                        if "no_attn" in _DBG:
                            continue
                        ek = ev = None
                        if use_enc:
                            # prefetch encoder K/V before the own attention
                            ek = p_enck.tile([72, _N], f32r, name=f"ek{i}",
                                             tag="enck")
                            nc.sync.dma_start(ek[:], enckT.ap()[:, i, :])
                            ev = p_encv.tile([128, _NT, 73], f32r,
                                             name=f"ev{i}", tag="encv")
                            nc.sync.dma_start(ev[:], encv.ap()[i])
                        xo = None
                        if use_own:
                            xo = p_xo.tile([73, _N], f32r, name=f"xo{i}",
                                           tag="xo")
                            attention(i, slots[2 * i + 1],
                                      lambda kb, i=i: vaug[:, kb, i, :],
                                      xo, f"own{i}")
                        ostar = None
                        if use_enc:
                            dt_enc = f32r if mode == "enc" else f32
                            ostar = p_ostar.tile([73, _N], dt_enc,
                                                 name=f"os{i}", tag="ostar")
                            attention(i, ek[:],
                                      lambda kb, ev=ev: ev[:, kb, :],
                                      ostar, f"enc{i}")
                        if use_loss and "no_loss" not in _DBG:
                            df = p_diff.tile([73, _N], f32, name=f"df{i}",
                                             tag="diff")
                            nc.vector.tensor_tensor(
                                out=df[:], in0=xo[:].bitcast(f32),
                                in1=ostar[:], op=OP.subtract)
                            sq = p_sq.tile([73, _N], f32, name=f"sq{i}",
                                           tag="sq")
                            nc.vector.tensor_tensor(
                                out=sq[:], in0=df[:], in1=df[:], op=OP.mult)
                            nc.vector.reduce_sum(
                                loss_sb[:, i:i + 1], sq[:],
                                axis=mybir.AxisListType.X)
                        # spill rows 1:73 packed at rows [72i, 72i+72)
                        spill = xo if use_own else ostar
                        c0 = 72 * i
                        for kt in range(c0 // 128, (c0 + 71) // 128 + 1):
                            lo = max(c0, 128 * kt)
                            hi = min(c0 + 72, 128 * kt + 128)
                            nc.sync.dma_start(
                                xod[lo - 128 * kt: hi - 128 * kt, kt, :],
                                spill[1 + lo - c0: 1 + hi - c0, :])

                nc.sync.dma_start(lossv.ap(), loss_sb[:])

            # ---- phase B: single-pass projection over the packed spill
            with ExitStack() as bctx:
                p_pw = bctx.enter_context(tc.tile_pool(name="p_pw", bufs=1))
                p_xor = bctx.enter_context(tc.tile_pool(name="p_xor", bufs=1))
                p_pout = bctx.enter_context(tc.tile_pool(name="p_pout", bufs=4))
                p_pps = bctx.enter_context(
                    tc.tile_pool(name="p_pps", bufs=3, space="PSUM"))
                _SCOPES.append((nc.next_id(), "proj"))
                xop = p_xor.tile([128, 5, _N], f32r, name="xop")
                pwp = p_pw.tile([128, 5, _D], f32r, name="pwp")
                for kt in range(5):
                    nc.sync.dma_start(xop[:, kt, :], xod[:, kt, :])
                    nc.sync.dma_start(pwp[:, kt, :], pw.ap()[:, kt, :])
                for ct in range(_KD):
                    pps = p_pps.tile([128, 1024], f32, name=f"pps{ct}",
                                     tag="pps")
                    for kt in range(5):
                        kk = 128 if kt < 4 else 64
                        for nch in range(2):
                            nc.tensor.matmul(
                                pps[:, nch * 512:(nch + 1) * 512],
                                pwp[0:kk, kt, ct * 128:(ct + 1) * 128],
                                xop[0:kk, kt, nch * 512:(nch + 1) * 512],
                                start=(kt == 0), stop=(kt == 4))
                    po = p_pout.tile([128, 1024], f32, name=f"po{ct}",
                                     tag="po")
                    for half in range(2):
                        sl = slice(half * 512, (half + 1) * 512)
                        if (ct + half) % 2 == 0:
                            nc.vector.tensor_copy(po[:, sl], pps[:, sl])
                        else:
                            nc.scalar.copy(po[:, sl], pps[:, sl])
                        nc.sync.dma_start(
                            outT.ap()[ct * 128:(ct + 1) * 128, sl], po[:, sl])

    nc.compile()
    return nc


def _prep_core(x, enc_k, enc_v, wT, qkv_b, pwT_aug, b, hg):
    """Host-side per-core input dict. wT = qkv_w.T (D, 3D); pwT_aug
    (H, 73, D) with zero row 0."""
    heads = range(hg * _NH, hg * _NH + _NH)
    qcols = np.concatenate(
        [np.arange(h * _HD, (h + 1) * _HD) for h in heads])
    # interleaved [q_h, k_h] channel order
    cols = np.empty(2 * len(qcols), np.int64)
    for idx, h in enumerate(heads):
        base = 144 * idx
        cols[base:base + 72] = np.arange(h * _HD, (h + 1) * _HD)
        cols[base + 72:base + 144] = _D + np.arange(h * _HD, (h + 1) * _HD)
    vcols = 2 * _D + qcols

    xTc = np.ascontiguousarray(x[b].T)
    wqk_c = np.ascontiguousarray(wT[:, cols])
    wv_c = np.ascontiguousarray(wT[:, vcols])
    bqk_c = np.ascontiguousarray(qkv_b[cols].reshape(_KD, 128).T)
    bvb_c = np.ascontiguousarray(
        np.broadcast_to(qkv_b[vcols], (128, 576)))
    ek = enc_k[b, list(heads)]                      # (NH, N, hd)
    enckT_c = np.ascontiguousarray(ek.transpose(2, 0, 1))   # (hd, NH, N)
    ev = enc_v[b, list(heads)].reshape(_NH, _NT, 128, _HD)
    encv_c = np.zeros((_NH, 128, _NT, 73), np.float32)
    encv_c[:, :, :, 0] = 1.0
    encv_c[:, :, :, 1:] = ev.transpose(0, 2, 1, 3)
    pw_c = pwT_aug[hg]
    return {
        "xT": xTc, "wqk": wqk_c, "wv": wv_c, "bqk": bqk_c, "bvb": bvb_c,
        "enckT": enckT_c, "encv": encv_c, "pw": pw_c,
    }



def build_in_maps(x, enc_k, enc_v, qkv_w, qkv_b, proj_w):
    """Host-side prep: per-core input dicts for all 8 cores."""
    x = np.asarray(x, np.float32)
    enc_k = np.asarray(enc_k, np.float32)
    enc_v = np.asarray(enc_v, np.float32)
    qkv_w = np.asarray(qkv_w, np.float32)
    qkv_b = np.asarray(qkv_b, np.float32)
    proj_w = np.asarray(proj_w, np.float32)
    wT = np.ascontiguousarray(qkv_w.T)
    pwT = np.asarray(proj_w.T)                       # (j, c)
    # packed per-core proj weights: (128, 5, D), rows = 8 heads x 72, zero pad
    pwT_aug = np.zeros((2, 640, _D), np.float32)
    for hg in range(2):
        pwT_aug[hg, :576] = pwT.reshape(_H, _HD, _D)[
            hg * _NH: hg * _NH + _NH].reshape(576, _D)
    pwT_aug = np.ascontiguousarray(
        pwT_aug.reshape(2, 5, 128, _D).transpose(0, 2, 1, 3))  # (2,128,5,D)
    return [
        _prep_core(x, enc_k, enc_v, wT, qkv_b, pwT_aug, c // 2, c % 2)
        for c in range(_NCORES)
    ]


LAST_EXEC_NS = None


def kernel(x, enc_k, enc_v, qkv_w, qkv_b, proj_w, proj_b, stage):
    global LAST_EXEC_NS
    from concourse.bass_utils import run_bass_kernel_spmd

    x = np.asarray(x, np.float32)
    enc_k = np.asarray(enc_k, np.float32)
    enc_v = np.asarray(enc_v, np.float32)
    qkv_w = np.asarray(qkv_w, np.float32)
    qkv_b = np.asarray(qkv_b, np.float32)
    proj_w = np.asarray(proj_w, np.float32)
    proj_b = np.asarray(proj_b, np.float32)
    stage = int(np.asarray(stage))
    mode = {1: "enc", 2: "both"}.get(stage, "own")

    if mode not in _CACHE:
        _CACHE[mode] = _build(mode)
    nc = _CACHE[mode]

    in_maps = build_in_maps(x, enc_k, enc_v, qkv_w, qkv_b, proj_w)
    res = run_bass_kernel_spmd(nc, in_maps, core_ids=list(range(_NCORES)))
    LAST_EXEC_NS = res.exec_time_ns

    out = np.empty((_B, _N, _D), np.float32)
    for b in range(_B):
        acc = res.results[2 * b]["outT"] + res.results[2 * b + 1]["outT"]
        out[b] = acc.T + proj_b

    if mode == "both":
        tot = sum(float(r["lossv"].sum()) for r in res.results)
        loss = np.float32(tot / (_B * _H * _N * _HD))
    else:
        loss = np.float32(0.0)
    return out, loss


# revision 29
# speedup vs baseline: 1.0810x; 1.0427x over previous
"""Trainium2 Bass kernel for AttentionWithEncoderKV (stage-2 distill attention).

Contract: kernel(**inputs) takes FULL unsharded numpy inputs and returns the
FULL output tuple (out, distill_loss), matching the jax reference:

    qkv = x @ qkv_w.T + qkv_b                       (B, N, 3, H, hd)
    x_out   = softmax(q*sc @ k^T) @ v               (own attention)
    o_star  = softmax(q*sc @ ek^T) @ ev             (encoder attention)
    loss    = mean((x_out - o_star)^2)              (stage 2 only)
    out     = x_out @ proj_w.T + proj_b

Sharding (8 cores): core c -> sample b = c//2, head-group hg = c%2 (8 of 16
heads).  Each core computes its heads' QKV, both attentions, a distill-loss
partial and a partial output projection (sum over its heads).  The host adds
the two per-sample projection partials (+bias) and the loss partials.

Device-side layout choices (all matmul operands at partition base 0):
 - host pre-transposes x, qkv_w, proj_w, enc_k so no on-device transposes.
 - scores are computed transposed (n_k on partitions) so P@V needs no
   transpose; V carries a leading ones-column so the PV matmul also produces
   softmax denominators in row 0; proj_w gets a matching zero row.
 - fp32r matmuls (full PE rate at free-dim >= 256, ~1.6e-4 component error).
 - per-head q^T/k^T slots are filled from the packed QKV output via
   SBUF->SBUF DMA (compute engines cannot shift partition bases; DMA can).
 - softmax denominators: DVE reciprocal of row 0, GPSIMD partition-broadcast,
   one DVE multiply; exp runs on ScalarE with the hd^-0.5 scale fused.
 - normalized per-head outputs spill to DRAM packed (128, 5, N) so the output
   projection runs one K=128-packed pass with 2 psum banks.
 - consecutive matmuls share their stationary operand (kt-outer/nch-inner
   loops) to halve the per-matmul self-weight-load overhead of fp32r.

Measured (8 trn2 cores, this container): relative error 4.1e-4 vs the fp32
jax reference (loss 2.4e-6); ~320-330 us per execution (hardware-loop slope
method); cost-model timeline 232 us with PE busy 183 us (79% occupancy).
"""

import math

import numpy as np

_B, _N, _D, _H, _HD = 4, 1024, 1152, 16, 72
_NH = _H // 2            # heads per core
_NT = _N // 128          # token tiles
_KD = _D // 128          # contraction d-tiles
_SC = _HD ** -0.5
_NCORES = 8

_CACHE = {}
_DBG = frozenset()  # debug kill-switches: no_attn, no_norm, no_loss, no_projB
_SCOPES = []        # (instruction-id watermark, label) markers for profiling


def _pieces_by_jt():
    """Split the 16 interleaved 72-channel half-slots (q0,k0,q1,k1,...) of the
    packed 1152-channel QKV output into per-128-tile contiguous pieces.

    Returns {jt: [(a, length, slot, o)]}: rows [a, a+length) of packed tile jt
    hold rows [o, o+length) of half-slot `slot`."""
    out = {jt: [] for jt in range(_KD)}
    for s in range(16):
        c0, c1 = 72 * s, 72 * s + 72
        jt0, jt1 = c0 // 128, (c1 - 1) // 128
        for jt in range(jt0, jt1 + 1):
            lo, hi = max(c0, 128 * jt), min(c1, 128 * jt + 128)
            out[jt].append((lo - 128 * jt, hi - lo, s, lo - c0))
    return out


def _build(mode, reps=1):
    """Build + compile the SPMD device program. mode: 'both' | 'own' | 'enc'.
    reps > 1 wraps the body in a hardware loop (timing builds only)."""
    import concourse.mybir as mybir
    import concourse.tile as tile
    from concourse import bacc

    f32 = mybir.dt.float32
    f32r = mybir.dt.float32r
    AF = mybir.ActivationFunctionType
    OP = mybir.AluOpType

    use_own = mode in ("own", "both")
    use_enc = mode in ("enc", "both")
    use_loss = mode == "both"

    nc = bacc.Bacc("TRN2", target_bir_lowering=False, debug=False,
                   num_devices=_NCORES)

    xT = nc.dram_tensor("xT", (_D, _N), f32r, kind="ExternalInput")
    wqk = nc.dram_tensor("wqk", (_D, 1152), f32r, kind="ExternalInput")
    wv = nc.dram_tensor("wv", (_D, 576), f32r, kind="ExternalInput")
    bqk = nc.dram_tensor("bqk", (128, _KD), f32, kind="ExternalInput")
    bvb = nc.dram_tensor("bvb", (128, 576), f32, kind="ExternalInput")
    enckT = nc.dram_tensor("enckT", (_HD, _NH, _N), f32r, kind="ExternalInput")
    encv = nc.dram_tensor("encv", (_NH, 128, _NT, 73), f32r, kind="ExternalInput")
    pw = nc.dram_tensor("pw", (128, 5, _D), f32r, kind="ExternalInput")
    outT = nc.dram_tensor("outT", (_D, _N), f32, kind="ExternalOutput")
    lossv = nc.dram_tensor("lossv", (73, _NH), f32, kind="ExternalOutput")

    pieces = _pieces_by_jt()
    # head i's slots complete at jt = i+1; emit its attention two j-tiles
    # later so the evac->repack chain stays off the critical path
    heads_done_at = {jt: [] for jt in range(_KD)}
    for i in range(_NH):
        heads_done_at[min((144 * i + 143) // 128 + 2, _KD - 1)].append(i)

    from contextlib import ExitStack

    with tile.TileContext(nc) as tc:
        rep_ctx = ExitStack()
        if reps > 1:
            rep_ctx.enter_context(tc.For_i(0, reps, 1))
        with rep_ctx, ExitStack() as ctx:
            p_dram = ctx.enter_context(
                tc.tile_pool(name="p_dram", bufs=1, space="DRAM"))
            xod = p_dram.tile([128, 5, _N], f32r, name="xod")
            with ExitStack() as actx:
                def pool(name, bufs, space="SBUF"):
                    return actx.enter_context(
                        tc.tile_pool(name=name, bufs=bufs, space=space))

                p_xt = pool("p_xt", 1)
                p_wv = pool("p_wv", 1)
                p_bias = pool("p_bias", 1)
                p_vaug = pool("p_vaug", 1)
                p_wqk = pool("p_wqk", 3)
                p_qkpk = pool("p_qkpk", 2)
                p_slot = pool("p_slot", 7)
                p_exp = pool("p_exp", 4)
                p_enck = pool("p_enck", 2)
                p_encv = pool("p_encv", 2)
                p_xo = pool("p_xo", 4)
                p_ostar = pool("p_ostar", 2)
                p_oev = pool("p_oev", 3)
                p_rr = pool("p_rr", 1)
                p_rb = pool("p_rb", 1)
                p_diff = pool("p_diff", 1)
                p_sq = pool("p_sq", 1)
                p_loss = pool("p_loss", 1)
                p_mm = pool("p_mm", 3, "PSUM")
                p_ops = pool("p_ops", 1, "PSUM")

                # ---- resident loads (split DMAs so compute can start early)
                bqk_sb = p_bias.tile([128, _KD], f32, name="bqk_sb", tag="bqk")
                nc.sync.dma_start(bqk_sb[:], bqk.ap())
                bvb_sb = p_bias.tile([128, 576], f32, name="bvb_sb", tag="bvb")
                nc.sync.dma_start(bvb_sb[:], bvb.ap())
                xt = p_xt.tile([128, _KD, _N], f32r, name="xt")
                xTr = xT.ap().rearrange("(kt p) n -> p kt n", p=128)
                wv_sb = p_wv.tile([128, _KD, 576], f32r, name="wv_sb")
                wvr = wv.ap().rearrange("(kt p) j -> p kt j", p=128)
                for kt in range(_KD):
                    nc.sync.dma_start(xt[:, kt, :], xTr[:, kt, :])
                    nc.sync.dma_start(wv_sb[:, kt, :], wvr[:, kt, :])

                loss_sb = p_loss.tile([73, _NH], f32, name="loss_sb")
                nc.gpsimd.memset(loss_sb[:], 0.0)

                # ---- v projection into aug layout [p][kb][h][1+hd], col 0 = 1
                _SCOPES.append((nc.next_id(), "v_phase"))
                vaug = p_vaug.tile([128, _NT, _NH, 73], f32r, name="vaug")
                # memset cannot produce f32r; write the ones column (col 0 of
                # every (kb, h) slab) as 0*x + 1 via tensor_scalar instead
                zs = p_bias.tile([128, 64], f32, name="zs", tag="zs")
                nc.vector.memset(zs[:], 0.0)
                nc.vector.tensor_scalar(
                    out=vaug[:, :, :, 0:1],
                    in0=zs[:].rearrange("p (a b c) -> p a b c", b=_NH, c=1),
                    scalar1=0.0, scalar2=1.0,
                    op0=OP.mult, op1=OP.add)
                # ---- per-head slot tiles (created lazily in piece order)
                slots = {}

                def slot_tile(s):
                    if s not in slots:
                        kind = "q" if s % 2 == 0 else "k"
                        slots[s] = p_slot.tile(
                            [72, _N], f32r, name=f"slot_{kind}{s // 2}",
                            tag="slot")
                    return slots[s]

                wslabs = {}

                def load_wslab(jt):
                    w = p_wqk.tile([128, _KD, 128], f32r,
                                   name=f"wslab{jt}", tag="wqk")
                    nc.sync.dma_start(
                        w[:],
                        wqk.ap().rearrange("(kt p) j -> p kt j", p=128)
                        [:, :, jt * 128:(jt + 1) * 128])
                    wslabs[jt] = w

                def emit_qk_jtile(jt):
                    wslab = wslabs.pop(jt)
                    qp = p_mm.tile([128, 1024], f32, name=f"qp{jt}", tag="mm")
                    for kt in range(_KD):
                        for nch in range(2):
                            nc.tensor.matmul(
                                qp[:, nch * 512:(nch + 1) * 512],
                                wslab[:, kt, :],
                                xt[:, kt, nch * 512:(nch + 1) * 512],
                                start=(kt == 0), stop=(kt == _KD - 1))
                    pk = p_qkpk.tile([128, 1024], f32r, name=f"pk{jt}",
                                     tag="qkpk")
                    nc.vector.tensor_scalar_add(pk[:], qp[:],
                                                bqk_sb[:, jt:jt + 1])
                    for (a, ln, s, o) in pieces[jt]:
                        nc.sync.dma_start(slot_tile(s)[o:o + ln, :],
                                          pk[a:a + ln, :])

                load_wslab(0)
                load_wslab(1)
                for nt in range(_NT):
                    if nt == 4:
                        _SCOPES.append((nc.next_id(), "qk_jt0"))
                        emit_qk_jtile(0)
                    vp = p_mm.tile([128, 1024], f32, name=f"vp{nt}", tag="mm")
                    for kt in range(_KD):
                        for half in range(2):
                            nc.tensor.matmul(
                                vp[:, half * 512: half * 512 + 288],
                                xt[:, kt, nt * 128:(nt + 1) * 128],
                                wv_sb[:, kt, half * 288:(half + 1) * 288],
                                start=(kt == 0), stop=(kt == _KD - 1))
                    for half in range(2):
                        dst = vp[:, half * 512: half * 512 + 288]
                        nc.vector.tensor_tensor(
                            out=vaug[:, nt, half * 4: half * 4 + 4, 1:73],
                            in0=dst.rearrange("p (h d) -> p h d", d=72),
                            in1=bvb_sb[:, half * 288:(half + 1) * 288]
                                .rearrange("p (h d) -> p h d", d=72),
                            op=OP.add)

                def attention(i, kT_ap, v_of_kb, out_tile, out_name_hint):
                    """Emit one attention (head i): scores^T, exp, PV(+denom),
                    normalize into out_tile (73, N)."""
                    _SCOPES.append((nc.next_id(), f"attn_{out_name_hint}"))
                    q = slots[2 * i]
                    ops_t = p_ops.tile([73, 1024], f32,
                                       name=f"o_{out_name_hint}", tag="o")
                    for kb in range(_NT):
                        sp = p_mm.tile([128, 1024], f32,
                                       name=f"s_{out_name_hint}_{kb}", tag="mm")
                        for nch in range(2):
                            nc.tensor.matmul(
                                sp[:, nch * 512:(nch + 1) * 512],
                                kT_ap[:, kb * 128:(kb + 1) * 128],
                                q[:, nch * 512:(nch + 1) * 512],
                                start=True, stop=True)
                        ex = p_exp.tile([128, 1024], f32r,
                                        name=f"e_{out_name_hint}_{kb}",
                                        tag="exp")
                        nc.scalar.activation(ex[:], sp[:], AF.Exp, scale=_SC)
                        for nch in range(2):
                            nc.tensor.matmul(
                                ops_t[:, nch * 512:(nch + 1) * 512],
                                v_of_kb(kb),
                                ex[:, nch * 512:(nch + 1) * 512],
                                start=(kb == 0), stop=(kb == _NT - 1))
                    if "no_norm" in _DBG:
                        nc.vector.tensor_copy(out_tile[:], ops_t[:])
                        return
                    oev = p_oev.tile([73, 1024], f32,
                                     name=f"oev_{out_name_hint}", tag="oev")
                    nc.vector.tensor_copy(oev[:], ops_t[:])
                    rr = p_rr.tile([1, 1024], f32,
                                   name=f"rr_{out_name_hint}", tag="rr")
                    nc.vector.reciprocal(rr[:], oev[0:1, :])
                    rb = p_rb.tile([73, 1024], f32,
                                   name=f"rb_{out_name_hint}", tag="rb")
                    nc.gpsimd.partition_broadcast(rb[:], rr[:])
                    nc.vector.tensor_tensor(
                        out=out_tile[:], in0=oev[:], in1=rb[:], op=OP.mult)

                # ---- packed qk projection tiles, interleaved with attention
                for jt in range(1, _KD):
                    _SCOPES.append((nc.next_id(), f"qk_jt{jt}"))
                    if jt + 1 < _KD:
                        load_wslab(jt + 1)
                    emit_qk_jtile(jt)

                    for i in heads_done_at[jt]:
                        if "no_attn" in _DBG:
                            continue
                        ek = ev = None
                        if use_enc:
                            # prefetch encoder K/V before the own attention
                            ek = p_enck.tile([72, _N], f32r, name=f"ek{i}",
                                             tag="enck")
                            nc.sync.dma_start(ek[:], enckT.ap()[:, i, :])
                            ev = p_encv.tile([128, _NT, 73], f32r,
                                             name=f"ev{i}", tag="encv")
                            nc.sync.dma_start(ev[:], encv.ap()[i])
                        xo = None
                        if use_own:
                            xo = p_xo.tile([73, _N], f32r, name=f"xo{i}",
                                           tag="xo")
                            attention(i, slots[2 * i + 1],
                                      lambda kb, i=i: vaug[:, kb, i, :],
                                      xo, f"own{i}")
                        ostar = None
                        if use_enc:
                            dt_enc = f32r if mode == "enc" else f32
                            ostar = p_ostar.tile([73, _N], dt_enc,
                                                 name=f"os{i}", tag="ostar")
                            attention(i, ek[:],
                                      lambda kb, ev=ev: ev[:, kb, :],
                                      ostar, f"enc{i}")
                        if use_loss and "no_loss" not in _DBG:
                            df = p_diff.tile([73, _N], f32, name=f"df{i}",
                                             tag="diff")
                            nc.vector.tensor_tensor(
                                out=df[:], in0=xo[:].bitcast(f32),
                                in1=ostar[:], op=OP.subtract)
                            sq = p_sq.tile([73, _N], f32, name=f"sq{i}",
                                           tag="sq")
                            nc.vector.tensor_tensor(
                                out=sq[:], in0=df[:], in1=df[:], op=OP.mult)
                            nc.vector.reduce_sum(
                                loss_sb[:, i:i + 1], sq[:],
                                axis=mybir.AxisListType.X)
                        # spill rows 1:73 packed at rows [72i, 72i+72)
                        spill = xo if use_own else ostar
                        c0 = 72 * i
                        for kt in range(c0 // 128, (c0 + 71) // 128 + 1):
                            lo = max(c0, 128 * kt)
                            hi = min(c0 + 72, 128 * kt + 128)
                            nc.sync.dma_start(
                                xod[lo - 128 * kt: hi - 128 * kt, kt, :],
                                spill[1 + lo - c0: 1 + hi - c0, :])

                nc.sync.dma_start(lossv.ap(), loss_sb[:])

            # ---- phase B: single-pass projection over the packed spill
            with ExitStack() as bctx:
                p_pw = bctx.enter_context(tc.tile_pool(name="p_pw", bufs=1))
                p_xor = bctx.enter_context(tc.tile_pool(name="p_xor", bufs=1))
                p_pout = bctx.enter_context(tc.tile_pool(name="p_pout", bufs=4))
                p_pps = bctx.enter_context(
                    tc.tile_pool(name="p_pps", bufs=3, space="PSUM"))
                _SCOPES.append((nc.next_id(), "proj"))
                xop = p_xor.tile([128, 5, _N], f32r, name="xop")
                pwp = p_pw.tile([128, 5, _D], f32r, name="pwp")
                for kt in range(5):
                    nc.sync.dma_start(xop[:, kt, :], xod[:, kt, :])
                    nc.sync.dma_start(pwp[:, kt, :], pw.ap()[:, kt, :])
                for ct in range(_KD):
                    pps = p_pps.tile([128, 1024], f32, name=f"pps{ct}",
                                     tag="pps")
                    for kt in range(5):
                        kk = 128 if kt < 4 else 64
                        for nch in range(2):
                            nc.tensor.matmul(
                                pps[:, nch * 512:(nch + 1) * 512],
                                pwp[0:kk, kt, ct * 128:(ct + 1) * 128],
                                xop[0:kk, kt, nch * 512:(nch + 1) * 512],
                                start=(kt == 0), stop=(kt == 4))
                    po = p_pout.tile([128, 1024], f32, name=f"po{ct}",
                                     tag="po")
                    for half in range(2):
                        sl = slice(half * 512, (half + 1) * 512)
                        if (ct + half) % 2 == 0:
                            nc.vector.tensor_copy(po[:, sl], pps[:, sl])
                        else:
                            nc.scalar.copy(po[:, sl], pps[:, sl])
                        nc.sync.dma_start(
                            outT.ap()[ct * 128:(ct + 1) * 128, sl], po[:, sl])

    nc.compile()
    return nc


def _prep_core(x, enc_k, enc_v, wT, qkv_b, pwT_aug, b, hg):
    """Host-side per-core input dict. wT = qkv_w.T (D, 3D); pwT_aug
    (H, 73, D) with zero row 0."""
    heads = range(hg * _NH, hg * _NH + _NH)
    qcols = np.concatenate(
        [np.arange(h * _HD, (h + 1) * _HD) for h in heads])
    # interleaved [q_h, k_h] channel order
    cols = np.empty(2 * len(qcols), np.int64)
    for idx, h in enumerate(heads):
        base = 144 * idx
        cols[base:base + 72] = np.arange(h * _HD, (h + 1) * _HD)
        cols[base + 72:base + 144] = _D + np.arange(h * _HD, (h + 1) * _HD)
    vcols = 2 * _D + qcols

    xTc = np.ascontiguousarray(x[b].T)
    wqk_c = np.ascontiguousarray(wT[:, cols])
    wv_c = np.ascontiguousarray(wT[:, vcols])
    bqk_c = np.ascontiguousarray(qkv_b[cols].reshape(_KD, 128).T)
    bvb_c = np.ascontiguousarray(
        np.broadcast_to(qkv_b[vcols], (128, 576)))
    ek = enc_k[b, list(heads)]                      # (NH, N, hd)
    enckT_c = np.ascontiguousarray(ek.transpose(2, 0, 1))   # (hd, NH, N)
    ev = enc_v[b, list(heads)].reshape(_NH, _NT, 128, _HD)
    encv_c = np.zeros((_NH, 128, _NT, 73), np.float32)
    encv_c[:, :, :, 0] = 1.0
    encv_c[:, :, :, 1:] = ev.transpose(0, 2, 1, 3)
    pw_c = pwT_aug[hg]
    return {
        "xT": xTc, "wqk": wqk_c, "wv": wv_c, "bqk": bqk_c, "bvb": bvb_c,
        "enckT": enckT_c, "encv": encv_c, "pw": pw_c,
    }



def build_in_maps(x, enc_k, enc_v, qkv_w, qkv_b, proj_w):
    """Host-side prep: per-core input dicts for all 8 cores."""
    x = np.asarray(x, np.float32)
    enc_k = np.asarray(enc_k, np.float32)
    enc_v = np.asarray(enc_v, np.float32)
    qkv_w = np.asarray(qkv_w, np.float32)
    qkv_b = np.asarray(qkv_b, np.float32)
    proj_w = np.asarray(proj_w, np.float32)
    wT = np.ascontiguousarray(qkv_w.T)
    pwT = np.asarray(proj_w.T)                       # (j, c)
    # packed per-core proj weights: (128, 5, D), rows = 8 heads x 72, zero pad
    pwT_aug = np.zeros((2, 640, _D), np.float32)
    for hg in range(2):
        pwT_aug[hg, :576] = pwT.reshape(_H, _HD, _D)[
            hg * _NH: hg * _NH + _NH].reshape(576, _D)
    pwT_aug = np.ascontiguousarray(
        pwT_aug.reshape(2, 5, 128, _D).transpose(0, 2, 1, 3))  # (2,128,5,D)
    return [
        _prep_core(x, enc_k, enc_v, wT, qkv_b, pwT_aug, c // 2, c % 2)
        for c in range(_NCORES)
    ]


LAST_EXEC_NS = None


def kernel(x, enc_k, enc_v, qkv_w, qkv_b, proj_w, proj_b, stage):
    global LAST_EXEC_NS
    from concourse.bass_utils import run_bass_kernel_spmd

    x = np.asarray(x, np.float32)
    enc_k = np.asarray(enc_k, np.float32)
    enc_v = np.asarray(enc_v, np.float32)
    qkv_w = np.asarray(qkv_w, np.float32)
    qkv_b = np.asarray(qkv_b, np.float32)
    proj_w = np.asarray(proj_w, np.float32)
    proj_b = np.asarray(proj_b, np.float32)
    stage = int(np.asarray(stage))
    mode = {1: "enc", 2: "both"}.get(stage, "own")

    if mode not in _CACHE:
        _CACHE[mode] = _build(mode)
    nc = _CACHE[mode]

    in_maps = build_in_maps(x, enc_k, enc_v, qkv_w, qkv_b, proj_w)
    res = run_bass_kernel_spmd(nc, in_maps, core_ids=list(range(_NCORES)))
    LAST_EXEC_NS = res.exec_time_ns

    out = np.empty((_B, _N, _D), np.float32)
    for b in range(_B):
        acc = res.results[2 * b]["outT"] + res.results[2 * b + 1]["outT"]
        out[b] = acc.T + proj_b

    if mode == "both":
        tot = sum(float(r["lossv"].sum()) for r in res.results)
        loss = np.float32(tot / (_B * _H * _N * _HD))
    else:
        loss = np.float32(0.0)
    return out, loss
